# revision 5
# baseline (speedup 1.0000x reference)
"""CARE-GNN Trainium2 kernel v2 (nn_CAREGNN_62199716381202).

Graph/data parallel over 8 NeuronCores, dst-sharded (6250 dsts/core).

vs v1 baseline:
- single-plane fp16 feature gathers (256B rows) for BOTH layers; fp32
  epilogue via fp32 x_own (no bf16 hi/lo reconstruction).
- layer-0 and layer-1 share identical idx/dst streams (same graph).
- gather calls aligned to (etype, block, half) groups with trailing -1
  padding: the SWDGE ucode trims trailing negative indices at runtime,
  so per-core padding descriptors are skipped even though the IR is
  shared across cores (SPMD).
- layer-1 one-hot matmuls stream only the 64 useful h columns.
- chunked AllGather (fp16) overlapped with layer-0 epilogue.
- epilogue windows of 4 blocks to batch DVE/ACT work.
"""

import os
import sys

if "/opt/trn_rl_repo" not in sys.path:
    sys.path.insert(0, "/opt/trn_rl_repo")

PADNEG = False
N_AG = 4
HAGDUMMY = False
TRIMREG = False
WINE = 4
HAGEXT = True
HAGSH = False
NQ = 4
SP = True

import numpy as np
import ml_dtypes

F16NP = np.float16

import concourse.bass as bass
import concourse.bacc as bacc
import concourse.mybir as mybir
import concourse.tile as tile
from concourse.bass_utils import run_bass_kernel_spmd

F32 = mybir.dt.float32
FP16 = mybir.dt.float16
I16 = mybir.dt.int16
ADD = mybir.AluOpType.add
MULT = mybir.AluOpType.mult
ISEQ = mybir.AluOpType.is_equal
TANH = mybir.ActivationFunctionType.Tanh


class Cfg:
    def __init__(self, N=50000, E=500000, n_cores=8, split=32768, win=None,
                 n_ag=None):
        n_ag = N_AG if n_ag is None else n_ag
        win = WINE if win is None else win
        self.N = N
        self.E = E
        self.D = 128
        self.HID = 64
        self.C = 2
        self.NET = 3
        self.n_cores = n_cores
        self.split = split
        self.win = win
        self.n_ag = n_ag
        assert N % n_cores == 0
        self.ND = N // n_cores
        self.NB = (self.ND + 127) // 128
        self.windows = [
            (s, min(win, self.NB - s)) for s in range(0, self.NB, win)
        ]

    def bs(self, b):
        return min(128, self.ND - b * 128)


def _wrap16(flat):
    assert flat.size % 16 == 0
    w = np.ascontiguousarray(flat.reshape(-1, 16).T).astype(np.int16)
    return np.tile(w, (8, 1))


def host_prep(cfg, inputs):
    """Build per-core input maps. Returns (in_maps, CA, CB)."""
    feat = np.asarray(inputs["feat"], np.float32)
    x16 = np.ascontiguousarray(feat.astype(F16NP))
    srcs = [np.asarray(inputs[f"src{i}"]) for i in range(cfg.NET)]
    dsts = [np.asarray(inputs[f"dst{i}"]) for i in range(cfg.NET)]

    # pass 1: per (core, etype, block, half) edge data -> CA, CB
    percore = []
    CA = CB = 1
    for k in range(cfg.n_cores):
        rows = []
        for i in range(cfg.NET):
            sel = (dsts[i] >= k * cfg.ND) & (dsts[i] < (k + 1) * cfg.ND)
            dl = (dsts[i][sel] - k * cfg.ND).astype(np.int64)
            s = srcs[i][sel].astype(np.int64)
            o = np.argsort(dl, kind="stable")
            dl, s = dl[o], s[o]
            b = dl >> 7
            half = (s >= cfg.split).astype(np.int64)
            grp = b * 2 + half
            o2 = np.argsort(grp, kind="stable")
            dl, s, b, half, grp = dl[o2], s[o2], b[o2], half[o2], grp[o2]
            if len(grp):
                newg = np.r_[True, grp[1:] != grp[:-1]]
                starts = np.flatnonzero(newg)
                lens = np.diff(np.r_[starts, len(grp)])
                cum = np.arange(len(grp)) - np.repeat(starts, lens)
                nA = np.max(np.where(half == 0, cum, -1)) + 1 if (half == 0).any() else 0
                nB = np.max(np.where(half == 1, cum, -1)) + 1 if (half == 1).any() else 0
            else:
                cum = np.zeros(0, np.int64)
                nA = nB = 0
            CA = max(CA, -(-int(nA) // 128))
            CB = max(CB, -(-int(nB) // 128))
            rows.append((dl, s, b, half, cum))
        percore.append(rows)

    LA = cfg.NET * cfg.NB * CA * 128
    LB = cfg.NET * cfg.NB * CB * 128
    GA = cfg.NET * cfg.NB * CA
    GB = cfg.NET * cfg.NB * CB

    # shared (replicated) small tensors
    Wm = np.asarray(inputs["Wm"], np.float32)
    bm = np.asarray(inputs["bm"], np.float32).reshape(cfg.C, 1)
    W0 = np.asarray(inputs["W0"], np.float32)
    b0 = np.asarray(inputs["b0"], np.float32).reshape(cfg.HID, 1)
    W1 = np.asarray(inputs["W1"], np.float32)
    b1 = np.asarray(inputs["b1"], np.float32).reshape(cfg.C, 1)
    p0 = np.tile(np.asarray(inputs["p0"], np.float32), (128, 1))
    p1 = np.tile(np.asarray(inputs["p1"], np.float32), (128, 1))
    gmax = cfg.win * max(CA, CB)
    iota = np.ascontiguousarray(
        np.broadcast_to(
            np.arange(128, dtype=np.float32).astype(F16NP)[None, :, None],
            (128, 128, gmax),
        ).reshape(128, 128 * gmax)
    )
    ident = np.eye(128, dtype=np.float32)

    in_maps = []
    for k in range(cfg.n_cores):
        # idx: -1 = trailing pad (runtime-trimmed per core); dst -1 = no match
        pad = -1 if PADNEG else 0
        idxA = np.full(LA, pad, np.int64)
        dlA = np.full(LA, -1.0, np.float32)
        idxB = np.full(LB, pad, np.int64)
        dlB = np.full(LB, -1.0, np.float32)
        icnt = np.ones((cfg.NET, cfg.NB * 128), np.float32)
        for i in range(cfg.NET):
            dl, s, b, half, cum = percore[k][i]
            cnt = np.bincount(dl, minlength=cfg.ND)
            icnt[i, : cfg.ND] = 1.0 / np.maximum(cnt, 1.0)
            mA = half == 0
            mB = half == 1
            posA = (i * cfg.NB + b[mA]) * CA * 128 + cum[mA]
            idxA[posA] = s[mA]
            dlA[posA] = dl[mA] - b[mA] * 128
            posB = (i * cfg.NB + b[mB]) * CB * 128 + cum[mB]
            idxB[posB] = s[mB] - cfg.split
            dlB[posB] = dl[mB] - b[mB] * 128
        x_own = np.zeros((cfg.NB * 128, cfg.D), np.float32)
        x_own[: cfg.ND] = feat[k * cfg.ND : (k + 1) * cfg.ND]
        # per-(etype, block, half) valid-idx counts, rounded up to 16
        cnts = np.zeros((cfg.NET * cfg.NB * 2,), np.int32)
        for i in range(cfg.NET):
            dl, s_, b, half, cum = percore[k][i]
            for hf in (0, 1):
                m = half == hf
                bc = np.bincount(b[m], minlength=cfg.NB)
                cnts[(i * cfg.NB + np.arange(cfg.NB)) * 2 + hf] = (
                    (bc + 15) // 16 * 16
                )
        in_maps.append(
            {
                "x16": x16,
                **({"hagdum": x16} if HAGDUMMY else {}),
                "x_own": x_own,
                "idxA": _wrap16(idxA),
                "idxB": _wrap16(idxB),
                "dstA": np.ascontiguousarray(
                    dlA.reshape(GA, 128).T
                ).astype(F16NP),
                "dstB": np.ascontiguousarray(
                    dlB.reshape(GB, 128).T
                ).astype(F16NP),
                "icnt": np.ascontiguousarray(
                    icnt.reshape(cfg.NET * cfg.NB, 128).T
                ),
                "cnts": cnts.reshape(1, -1),
                "Wm": Wm, "bm": bm, "W0": W0, "b0": b0, "W1": W1, "b1": b1,
                "p0": p0, "p1": p1, "iota": iota, "ident": ident,
            }
        )
    return in_maps, CA, CB


def build_nc(cfg, CA, CB, debug=False):
    N, ND, NB, NET, HID, C = cfg.N, cfg.ND, cfg.NB, cfg.NET, cfg.HID, cfg.C
    SPLIT = cfg.split
    WIN = cfg.win
    LA = NET * NB * CA * 128
    LB = NET * NB * CB * 128
    GA = NET * NB * CA
    GB = NET * NB * CB
    GMAX = WIN * max(CA, CB)

    nc = bacc.Bacc(trn_type="TRN2", num_devices=cfg.n_cores,
                   num_swdge_queues=4)

    x16_d = nc.dram_tensor("x16", [N, 128], FP16, kind="ExternalInput")
    hagdum_d = (nc.dram_tensor("hagdum", [N, 128], FP16, kind="ExternalInput")
                if HAGDUMMY else None)
    x_own_d = nc.dram_tensor("x_own", [NB * 128, 128], F32,
                             kind="ExternalInput")
    idxA_d = nc.dram_tensor("idxA", [128, LA // 16], I16, kind="ExternalInput")
    idxB_d = nc.dram_tensor("idxB", [128, LB // 16], I16, kind="ExternalInput")
    dstA_d = nc.dram_tensor("dstA", [128, GA], FP16, kind="ExternalInput")
    dstB_d = nc.dram_tensor("dstB", [128, GB], FP16, kind="ExternalInput")
    icnt_d = nc.dram_tensor("icnt", [128, NET * NB], F32, kind="ExternalInput")
    cnts_d = nc.dram_tensor("cnts", [1, NET * NB * 2], mybir.dt.int32,
                            kind="ExternalInput")
    Wm_d = nc.dram_tensor("Wm", [128, C], F32, kind="ExternalInput")
    bm_d = nc.dram_tensor("bm", [C, 1], F32, kind="ExternalInput")
    W0_d = nc.dram_tensor("W0", [128, HID], F32, kind="ExternalInput")
    b0_d = nc.dram_tensor("b0", [HID, 1], F32, kind="ExternalInput")
    W1_d = nc.dram_tensor("W1", [HID, C], F32, kind="ExternalInput")
    b1_d = nc.dram_tensor("b1", [C, 1], F32, kind="ExternalInput")
    p0_d = nc.dram_tensor("p0", [128, NET], F32, kind="ExternalInput")
    p1_d = nc.dram_tensor("p1", [128, NET], F32, kind="ExternalInput")
    iota_d = nc.dram_tensor("iota", [128, 128 * GMAX], FP16,
                            kind="ExternalInput")
    ident_d = nc.dram_tensor("ident", [128, 128], F32, kind="ExternalInput")
    outT_d = nc.dram_tensor("outT", [C, ND], F32, kind="ExternalOutput")
    hag_ext_d = (nc.dram_tensor("h_ag_ext", [N, 128], FP16,
                                kind="ExternalOutput") if HAGEXT else None)
    hag_sh_d = (nc.dram_tensor("h_ag_sh", [N, 128], FP16, kind="Internal",
                               addr_space="Shared") if HAGSH else None)
    simT_d = nc.dram_tensor("simT", [C, ND], F32, kind="ExternalOutput")

    nw = len(cfg.windows)
    ag_after = set()
    if cfg.n_ag >= 4 and nw >= 8:
        # front-loaded chunks: tiny tail so layer-1 unblocks early
        ag_after = {nw // 2 - 1, (3 * nw) // 4 - 1, nw - 3, nw - 2, nw - 1}
    else:
        for c in range(1, cfg.n_ag + 1):
            ag_after.add(min(nw - 1, (c * nw) // cfg.n_ag - 1))

    with tile.TileContext(nc) as tc:
        with (
            tc.tile_pool(name="const", bufs=1) as cp,
            tc.tile_pool(name="big", bufs=1) as bigp,
            tc.tile_pool(name="gath", bufs=2) as gp,
            tc.tile_pool(name="sgen", bufs=2) as sp,
            tc.tile_pool(name="work", bufs=2) as wp,
            tc.tile_pool(name="ps", bufs=2, space="PSUM") as pp,
            tc.tile_pool(name="pt", bufs=2, space="PSUM") as ptp,
            tc.tile_pool(name="po", bufs=2, space="PSUM") as pop,
            tc.tile_pool(name="dram", bufs=1, space="DRAM") as dp,
        ):
            # ---- resident constants / streams ----
            idxA = cp.tile([128, LA // 16], I16)
            idxB = cp.tile([128, LB // 16], I16)
            nc.sync.dma_start(out=idxA[:, :], in_=idxA_d[:, :])
            nc.sync.dma_start(out=idxB[:, :], in_=idxB_d[:, :])
            dstA = cp.tile([128, GA], FP16)
            dstB = cp.tile([128, GB], FP16)
            nc.sync.dma_start(out=dstA[:], in_=dstA_d[:, :])
            nc.sync.dma_start(out=dstB[:], in_=dstB_d[:, :])
            icnt = cp.tile([128, NET * NB], F32)
            nc.sync.dma_start(out=icnt[:], in_=icnt_d[:, :])
            cnts_s = cp.tile([1, NET * NB * 2], mybir.dt.int32)
            nc.sync.dma_start(out=cnts_s[:, :], in_=cnts_d[:, :])
            gcnt_reg = (nc.gpsimd.alloc_register("gcnt")
                        if TRIMREG else None)
            Wm_s = cp.tile([128, C], F32)
            bm_s = cp.tile([C, 1], F32)
            W0_s = cp.tile([128, HID], F32)
            b0_s = cp.tile([HID, 1], F32)
            W1_s = cp.tile([HID, C], F32)
            b1_s = cp.tile([C, 1], F32)
            p0_s = cp.tile([128, NET], F32)
            p1_s = cp.tile([128, NET], F32)
            iota_s = cp.tile([128, 128, GMAX], FP16)
            ident_s = cp.tile([128, 128], F32)
            for t_, d_ in [
                (Wm_s, Wm_d), (bm_s, bm_d), (W0_s, W0_d), (b0_s, b0_d),
                (W1_s, W1_d), (b1_s, b1_d), (p0_s, p0_d), (p1_s, p1_d),
                (iota_s, iota_d.rearrange("p (j g) -> p j g", j=128)),
                (ident_s, ident_d),
            ]:
                nc.sync.dma_start(out=t_[:], in_=d_[:, :])

            hacc = bigp.tile([128, NB, 128], F32)   # layer-0 weighted agg
            h1acc = bigp.tile([128, NB, HID], F32)  # layer-1 weighted agg
            hnat = bigp.tile([128, NB, HID], F32)   # layer-0 output (natural)

            h_loc = dp.tile([ND, 128], FP16)
            h_stage = dp.tile([N, 128], FP16)  # chunk-major AG landing
            if HAGSH:
                h_ag = hag_sh_d
                ag_dst = h_ag
            elif HAGEXT:
                h_ag = hag_ext_d
                ag_dst = h_stage  # collective may not write IO tensors
            else:
                h_ag = dp.tile([N, 128], FP16)  # core-major gather table
                ag_dst = h_ag
            h_ag_v = h_ag[:, :].rearrange("(k r) d -> k r d", k=cfg.n_cores)
            ag_dst_v = ag_dst[:, :].rearrange(
                "(k r) d -> k r d", k=cfg.n_cores
            )

            qctr = [0]

            # pre-zero gather rings so stale (trimmed) slots never hold NaN
            for tg in ("gA0", "gB0", "gA1", "gB1"):
                cw = CA if "A" in tg else CB
                for _ in range(2):
                    g = gp.tile([128, WIN * cw, 128], FP16, tag=tg)
                    nc.vector.memset(g[:, :, :], 0)

            def agg_window(layer, src_lo, src_hi, ecols, acc, p_s, b0w, wb):
                """Aggregate all etypes for one window of dst blocks."""
                for i in range(NET):
                    gc0A = (i * NB + b0w) * CA
                    gc0B = (i * NB + b0w) * CB
                    gA = gp.tile([128, WIN * CA, 128], FP16, tag=f"gA{layer}")
                    gB = gp.tile([128, WIN * CB, 128], FP16, tag=f"gB{layer}")
                    # one gather call per (etype, block, half): all padding
                    # is trailing -1 -> trimmed at runtime per core
                    CHC = 8  # chunks per gather call
                    for bb in range(wb):
                        c0 = 0
                        while c0 < CA:
                            cc = min(CHC, CA - c0)
                            q = qctr[0] % NQ
                            qctr[0] += 1
                            cid = ((i * NB + b0w + bb) * 2 + 0)
                            if TRIMREG:
                                nc.gpsimd.reg_load(
                                    gcnt_reg, cnts_s[0:1, cid : cid + 1]
                                )
                                nreg = gcnt_reg
                            else:
                                nreg = cc * 128
                            nc.gpsimd.dma_gather(
                                gA[:, bb * CA + c0 : bb * CA + c0 + cc, :],
                                src_lo,
                                idxA[:, (gc0A + bb * CA + c0) * 8
                                     : (gc0A + bb * CA + c0 + cc) * 8],
                                cc * 128, nreg, 128, queue_num=q,
                            )
                            c0 += cc
                        c0 = 0
                        while c0 < CB:
                            cc = min(CHC, CB - c0)
                            q = qctr[0] % NQ
                            qctr[0] += 1
                            cid = ((i * NB + b0w + bb) * 2 + 1)
                            if TRIMREG:
                                nc.gpsimd.reg_load(
                                    gcnt_reg, cnts_s[0:1, cid : cid + 1]
                                )
                                nreg = gcnt_reg
                            else:
                                nreg = cc * 128
                            nc.gpsimd.dma_gather(
                                gB[:, bb * CB + c0 : bb * CB + c0 + cc, :],
                                src_hi,
                                idxB[:, (gc0B + bb * CB + c0) * 8
                                     : (gc0B + bb * CB + c0 + cc) * 8],
                                cc * 128, nreg, 128, queue_num=q,
                            )
                            c0 += cc
                    SA = sp.tile([128, 128, WIN * CA], FP16, tag="SA")
                    SB = sp.tile([128, 128, WIN * CB], FP16, tag="SB")
                    nc.vector.tensor_tensor(
                        SA[:, :, : wb * CA],
                        iota_s[:, :, : wb * CA],
                        dstA[:, gc0A : gc0A + wb * CA].unsqueeze(
                            1
                        ).broadcast_to([128, 128, wb * CA]),
                        ISEQ,
                    )
                    nc.vector.tensor_tensor(
                        SB[:, :, : wb * CB],
                        iota_s[:, :, : wb * CB],
                        dstB[:, gc0B : gc0B + wb * CB].unsqueeze(
                            1
                        ).broadcast_to([128, 128, wb * CB]),
                        ISEQ,
                    )
                    ps = pp.tile([128, WIN, 128], F32, tag="ps")
                    for bb in range(wb):
                        for c in range(CA):
                            nc.tensor.matmul(
                                ps[:, bb, 0:ecols],
                                SA[:, :, bb * CA + c],
                                gA[:, bb * CA + c, 0:ecols],
                                start=(c == 0),
                                stop=False,
                            )
                        for c in range(CB):
                            nc.tensor.matmul(
                                ps[:, bb, 0:ecols],
                                SB[:, :, bb * CB + c],
                                gB[:, bb * CB + c, 0:ecols],
                                start=False,
                                stop=(c == CB - 1),
                            )
                    hr = wp.tile([128, WIN, ecols], F32, tag=f"hr{layer}")
                    for bb in range(wb):
                        nc.scalar.activation(
                            hr[:, bb, :], ps[:, bb, 0:ecols], TANH,
                            scale=icnt[:, i * NB + b0w + bb
                                       : i * NB + b0w + bb + 1],
                        )
                    accs = acc[:, b0w : b0w + wb, 0:ecols]
                    if i == 0:
                        nc.scalar.mul(accs, hr[:, :wb, :], p_s[:, 0:1])
                    else:
                        tmp = wp.tile([128, WIN, ecols], F32, tag=f"tm{layer}")
                        nc.scalar.mul(
                            tmp[:, :wb, :], hr[:, :wb, :], p_s[:, i : i + 1]
                        )
                        nc.vector.tensor_tensor(
                            accs, accs, tmp[:, :wb, :], ADD
                        )

            # ================= LAYER 0 =================
            ag_row0 = [0]
            for wi, (b0w, wb) in enumerate(cfg.windows):
                agg_window(0, x16_d[0:SPLIT, :], x16_d[SPLIT:N, :],
                           128, hacc, p0_s, b0w, wb)
                # epilogue: residual + W0 + sim + h rows
                fx = wp.tile([128, WIN, 128], F32, tag="fx")
                nc.sync.dma_start(
                    out=fx[:, :wb, :],
                    in_=x_own_d[b0w * 128 : (b0w + wb) * 128, :].rearrange(
                        "(b p) d -> p b d", p=128
                    ),
                )
                h0 = wp.tile([128, WIN, 128], F32, tag="h0")
                nc.vector.tensor_tensor(
                    h0[:, :wb, :], hacc[:, b0w : b0w + wb, :],
                    fx[:, :wb, :], ADD,
                )
                nc.scalar.activation(h0[:, :wb, :], h0[:, :wb, :], TANH)
                so = wp.tile([C, WIN * 128], F32, tag="so")
                for bb in range(wb):
                    b = b0w + bb
                    n = cfg.bs(b)
                    ptt = ptp.tile([128, 128], F32, tag="ptt")
                    nc.tensor.transpose(ptt[:], h0[:, bb, :], ident_s[:])
                    hT = wp.tile([128, 128], F32, tag="hT")
                    nc.vector.tensor_copy(hT[:], ptt[:])
                    po = pop.tile([HID, 128], F32, tag="po")
                    nc.tensor.matmul(po[:], W0_s[:], hT[:])
                    o_sb = wp.tile([HID, 128], F32, tag="osb")
                    nc.vector.tensor_scalar(
                        o_sb[:], po[:], b0_s[:, 0:1], None, ADD
                    )
                    pt2 = ptp.tile([128, HID], F32, tag="ptt")
                    nc.tensor.transpose(
                        pt2[:], o_sb[:], ident_s[0:HID, 0:HID]
                    )
                    nc.vector.tensor_copy(hnat[:, b, :], pt2[:])
                    hl = wp.tile([128, 128], FP16, tag="hl")
                    nc.vector.memset(hl[:, HID:128], 0)
                    nc.vector.tensor_copy(hl[:, 0:HID], pt2[:])
                    nc.sync.dma_start(
                        out=h_loc[b * 128 : b * 128 + n, :],
                        in_=hl[0:n, :],
                    )
                    # sim = tanh(feat @ Wm + bm)
                    ptf = ptp.tile([128, 128], F32, tag="ptt")
                    nc.tensor.transpose(ptf[:], fx[:, bb, :], ident_s[:])
                    fT = wp.tile([128, 128], F32, tag="fT")
                    nc.vector.tensor_copy(fT[:], ptf[:])
                    psim = pop.tile([C, 128], F32, tag="po")
                    nc.tensor.matmul(psim[:], Wm_s[:], fT[:])
                    nc.scalar.activation(
                        so[:, bb * 128 : (bb + 1) * 128], psim[:], TANH,
                        bias=bm_s[:, 0:1],
                    )
                wcols = min(wb * 128, ND - b0w * 128)
                nc.sync.dma_start(
                    out=simT_d[:, b0w * 128 : b0w * 128 + wcols],
                    in_=so[:, 0:wcols],
                )
                # chunked AllGather of finished h rows
                if wi in ag_after:
                    r0 = ag_row0[0]
                    r1 = min(ND, (b0w + wb) * 128)
                    if r1 > r0:
                        K = cfg.n_cores
                        if cfg.n_ag == 1:
                            nc.gpsimd.collective_compute(
                                "AllGather",
                                mybir.AluOpType.bypass,
                                replica_groups=[list(range(K))],
                                ins=[h_loc[r0:r1, :].opt()],
                                outs=[ag_dst_v[:, r0:r1, :].opt()],
                            )
                            if HAGEXT:
                                nc.sync.dma_start(
                                    out=h_ag[:, :], in_=ag_dst[:, :]
                                )
                        else:
                            stg = h_stage[K * r0 : K * r1, :].rearrange(
                                "(k r) d -> k r d", k=K
                            )
                            nc.gpsimd.collective_compute(
                                "AllGather",
                                mybir.AluOpType.bypass,
                                replica_groups=[list(range(K))],
                                ins=[h_loc[r0:r1, :].opt()],
                                outs=[stg.opt()],
                            )
                            # rearrange chunk-major -> core-major table
                            nc.sync.dma_start(out=h_ag_v[:, r0:r1, :], in_=stg)
                    ag_row0[0] = r1

            if HAGEXT or HAGSH:
                tc.strict_bb_all_engine_barrier()

            # ================= LAYER 1 =================
            l1src = hagdum_d if HAGDUMMY else h_ag
            for (b0w, wb) in cfg.windows:
                agg_window(1, l1src[0:SPLIT, :], l1src[SPLIT:N, :],
                           HID, h1acc, p1_s, b0w, wb)
                h2 = wp.tile([128, WIN, HID], F32, tag="h2")
                nc.vector.tensor_tensor(
                    h2[:, :wb, :], h1acc[:, b0w : b0w + wb, :],
                    hnat[:, b0w : b0w + wb, :], ADD,
                )
                nc.scalar.activation(h2[:, :wb, :], h2[:, :wb, :], TANH)
                oo = wp.tile([C, WIN * 128], F32, tag="oo")
                for bb in range(wb):
                    pt3 = ptp.tile([HID, 128], F32, tag="ptt")
                    nc.tensor.transpose(pt3[:], h2[:, bb, :], ident_s[:])
                    h2T = wp.tile([HID, 128], F32, tag="h2T")
                    nc.vector.tensor_copy(h2T[:], pt3[:])
                    po2 = pop.tile([C, 128], F32, tag="po")
                    nc.tensor.matmul(po2[:], W1_s[:], h2T[:])
                    nc.vector.tensor_scalar(
                        oo[:, bb * 128 : (bb + 1) * 128], po2[:],
                        b1_s[:, 0:1], None, ADD,
                    )
                wcols = min(wb * 128, ND - b0w * 128)
                nc.sync.dma_start(
                    out=outT_d[:, b0w * 128 : b0w * 128 + wcols],
                    in_=oo[:, 0:wcols],
                )

    nc.compile()
    return nc


_CACHE = {}


def _get_nc(cfg, CA, CB):
    key = (cfg.N, cfg.E, cfg.n_cores, CA, CB)
    if key not in _CACHE:
        _CACHE[key] = build_nc(cfg, CA, CB)
    return _CACHE[key]


def kernel(**inputs):
    cfg = Cfg()
    in_maps, CA, CB = host_prep(cfg, inputs)
    nc = _get_nc(cfg, CA, CB)
    res = run_bass_kernel_spmd(nc, in_maps, core_ids=list(range(cfg.n_cores)))
    out = np.concatenate(
        [r["outT"] for r in res.results], axis=1
    ).T.astype(np.float32)
    sim = np.concatenate(
        [r["simT"] for r in res.results], axis=1
    ).T.astype(np.float32)
    return (np.ascontiguousarray(out), np.ascontiguousarray(sim))
